# revision 22
# baseline (speedup 1.0000x reference)
"""Trainium2 Bass kernel for Longformer self-attention (B=2, S=4096, D=768, H=12, HD=64, W=256, G=32).

Sharding: 8 cores = 2 batches x 4 head-groups (3 heads each). Each core computes its
batch's projections restricted to its 192 output channels, runs banded + global
attention for its 3 heads, and returns an unnormalized transposed output
([3, 65, S]: rows 0-63 = head-dim, row 64 = softmax denominator z) plus the raw
global-query output [G, 3, 65]; the host divides by z, transposes, and assembles.

Key layout/scheduling choices (v2):
  - hidden_states pre-transposed on host -> plain contiguous DMA loads.
  - 5 projection chains of full 128 output rows (q01, k01, kg01, [q2|kg2],
    [k2|qg2]) -- no half-wasted 64-row chains.
  - band scores / sg / global scores run in row-tiled PE modes; instruction
    order groups matmuls by PE tiling mode (128x128 vs 64-row modes) per
    chunk to minimize array-reconfig drains, and alternates row-tiles
    (heads 0/2 on partitions 0:64 = tile T0, head 1 on 64:128 = T8) so
    independent tiles run concurrently.
  - global-key PV contribution stays in 128-mode via zero-padded operands
    (exp_sg rows 96:128 = 0, per-head global-v at partitions 32h:32h+32).
  - band mask applied as a single bf16 multiply on the exp'd probs (SBUF),
    not f32 adds on PSUM.
  - v/vg bias via pre-broadcast bf16 add fused into the PSUM->SBUF copy.
  - PV outputs DMA'd to DRAM directly from PSUM.
Matmul inputs bf16, fp32 PSUM/softmax.
"""
import numpy as np
import ml_dtypes

import concourse.bass as bass
import concourse.mybir as mybir
import concourse.tile as tile
from concourse import bacc
from concourse.bass_utils import run_bass_kernel_spmd

B, S, D, H, HD = 2, 4096, 768, 12, 64
W = 256
G = 32
SCALE = 1.0 / np.float32(np.sqrt(HD))
KB = 128
NKB = S // KB     # 32
QSB = 512
NQSB = S // QSB   # 8
NKT = D // 128    # 6
NNT = S // 512    # 8

BF = mybir.dt.bfloat16
F32 = mybir.dt.float32
AF = mybir.ActivationFunctionType
bf16 = ml_dtypes.bfloat16

_cache = {}


def _span(kb):
    k0 = KB * kb
    qlo, qhi = max(0, k0 - 2 * KB), min(S, k0 + 3 * KB)
    return qlo, qhi, qlo - (k0 - 2 * KB), qhi - (k0 - 2 * KB)


def _build():
    nc = bacc.Bacc(None, target_bir_lowering=False)

    hsT_d = nc.declare_dram_parameter("hsT", [128, NNT, NKT, 512], BF, isOutput=False)
    w5_d = nc.declare_dram_parameter("w5", [128, NKT, 5, 128], BF, isOutput=False)
    wqg_d = nc.declare_dram_parameter("wqg01", [128, NKT, 128], BF, isOutput=False)
    wvvg_d = nc.declare_dram_parameter("wvvg", [128, NKT, 384], BF, isOutput=False)
    bvvg_d = nc.declare_dram_parameter("bvvg", [1, 384], BF, isOutput=False)
    bias_d = nc.declare_dram_parameter("bias_t", [128, 8], F32, isOutput=False)
    masks_d = nc.declare_dram_parameter("masks", [128, 2, 128], BF, isOutput=False)
    id96_d = nc.declare_dram_parameter("id96", [96, 96], BF, isOutput=False)
    out_d = nc.declare_dram_parameter("out", [3, 65, S], F32, isOutput=True)
    outg_d = nc.declare_dram_parameter("outg", [3, G, 65], F32, isOutput=True)

    with tile.TileContext(nc) as tc:
        with tc.tile_pool(name="persist", bufs=1) as pp:
            masks_t = pp.tile([128, 2, 128], BF)
            ones_t = pp.tile([1, 128], BF)
            nc.vector.memset(ones_t[:], 1.0)

            qT01 = pp.tile([128, S], BF)
            qT2 = pp.tile([128, S], BF)      # rows 0-63 used (head 2 q)
            kT01 = pp.tile([128, S], BF)
            kT2 = pp.tile([128, S], BF)      # rows 0-63 used (head 2 k)
            v_nat = pp.tile([128, NKB, 3, 65], BF)
            # exp_sg: rows 32h..32h+31 = head h's exp'd global-key scores;
            # rows 96:128 stay zero so K=128 matmuls vs vGp are exact.
            exp_sg = pp.tile([128, S], BF)
            nc.vector.memset(v_nat[:, :, :, 64:65], 1.0)
            nc.vector.memset(exp_sg[96:128, :], 0.0)

            def sl(t01, t2, h):
                return t01[64 * h:64 * h + 64] if h < 2 else t2[0:64]

            with tc.tile_pool(name="ac", bufs=1) as ac:
                kgT01 = ac.tile([128, S], BF)
                kgT2 = ac.tile([128, S], BF)  # rows 64-127 used (head 2 kg)
                qgT01 = ac.tile([128, G], BF)
                qgT2 = ac.tile([128, G], BF)  # rows 64-127 used (head 2 qg)
                vg_nat = ac.tile([128, NKB, 3, 65], BF)
                vGp = ac.tile([128, 3, 65], BF)   # head h global-v at rows 32h:32h+32
                probs_g = ac.tile([96, S], BF)
                pb_gT = ac.tile([128, NKB, 96], BF)
                id96_t = ac.tile([96, 96], BF)
                bvvg_b = ac.tile([128, 384], BF)  # bias broadcast over tokens
                nc.vector.memset(vg_nat[:, :, :, 64:65], 1.0)
                nc.vector.memset(vGp[:], 0.0)
                for h in range(3):
                    nc.vector.memset(vGp[32 * h:32 * h + 32, h, 64:65], 1.0)

                def slg(h):
                    if h < 2:
                        return qgT01[64 * h:64 * h + 64], kgT01[64 * h:64 * h + 64]
                    return qgT2[64:128], kgT2[64:128]

                with (
                    tc.tile_pool(name="aw", bufs=1) as aw,
                    tc.tile_pool(name="hst", bufs=3) as hstp,
                    tc.tile_pool(name="upsum", bufs=4, space="PSUM") as upsum,
                    tc.tile_pool(name="pbt", bufs=30) as pbtp,
                    tc.tile_pool(name="osb", bufs=3) as osbp,
                ):
                    w5_t = aw.tile([128, NKT, 5, 128], BF)
                    wqg_t = aw.tile([128, NKT, 128], BF)
                    wvvg_t = aw.tile([128, NKT, 384], BF)
                    bvvg_t = aw.tile([1, 384], BF)
                    bias_t = aw.tile([128, 8], F32)
                    # issue startup DMAs from several engines in parallel;
                    # w5 + first hs chunk are on the critical path
                    nc.sync.dma_start(w5_t[:], w5_d[:])
                    hst0 = hstp.tile([128, NKT, 512], BF)
                    nc.sync.dma_start(hst0[:], hsT_d[:, 0, :, :])
                    nc.scalar.dma_start(wvvg_t[:], wvvg_d[:])
                    nc.scalar.dma_start(bias_t[:], bias_d[:])
                    nc.gpsimd.dma_start(wqg_t[:], wqg_d[:])
                    nc.gpsimd.dma_start(bvvg_t[:], bvvg_d[:])
                    nc.gpsimd.dma_start(masks_t[:], masks_d[:])
                    nc.gpsimd.dma_start(id96_t[:], id96_d[:])

                    pbt = {}

                    def mm_score(t, h, kb, a, b2):
                        k0 = KB * kb
                        qlo, qhi, llo, lhi = _span(kb)
                        nc.tensor.matmul(
                            t[:, a:b2],
                            sl(kT01, kT2, h)[:, k0:k0 + KB],
                            sl(qT01, qT2, h)[:, qlo + (a - llo):qlo + (a - llo) + (b2 - a)])

                    def exp_mask(ps, kb, h):
                        qlo, qhi, llo, lhi = _span(kb)
                        t_ = pbtp.tile([128, 640], BF, tag="pb")
                        nc.scalar.activation(t_[:, llo:lhi], ps[:, llo:lhi], AF.Exp)
                        # zero the out-of-window triangles (cols 0:128 and
                        # 512:640) with one strided bf16 multiply on GpSimd
                        tv = t_.rearrange("p (o j) -> p o j", o=5)
                        if llo == 0 and lhi == 640:
                            nc.vector.tensor_mul(tv[:, 0:5:4, :], tv[:, 0:5:4, :],
                                                 masks_t[:])
                        elif llo == 0:
                            nc.vector.tensor_mul(tv[:, 0, :], tv[:, 0, :],
                                                 masks_t[:, 0, :])
                        else:
                            nc.vector.tensor_mul(tv[:, 4, :], tv[:, 4, :],
                                                 masks_t[:, 1, :])
                        pbt[(kb, h)] = t_

                    def do_kb(kb):
                        qlo, qhi, llo, lhi = _span(kb)
                        pieces = [(a, b) for (a, b) in
                                  [(llo, min(lhi, 512)), (max(llo, 512), lhi)] if a < b]
                        tiles = []
                        for h in range(3):
                            tiles.append(upsum.tile([128, 640], F32, tag="u",
                                                    name=f"sc{kb}_{h}"))
                        # h0 on PE row-tile T0, h1 on T8: interleave their pieces
                        # so the two tiles run concurrently; h2 (T0) goes last so
                        # its PSUM slot (shared with h0) is free by then.
                        first = [0, 1] if kb % 2 == 0 else [1, 0]
                        for (a, b2) in pieces:
                            for h in first:
                                mm_score(tiles[h], h, kb, a, b2)
                        for h in first:
                            exp_mask(tiles[h], kb, h)
                        for (a, b2) in pieces:
                            mm_score(tiles[2], 2, kb, a, b2)
                        exp_mask(tiles[2], kb, 2)

                    def do_pv(qs):
                        q0 = QSB * qs
                        kbs = list(range(max(0, 4 * qs - 2), min(NKB, 4 * qs + 6)))
                        for h in range(3):
                            po = upsum.tile([128, 640], F32, tag="u")
                            nc.tensor.matmul(po[0:65, 0:512], vGp[:, h, :],
                                             exp_sg[:, q0:q0 + 512],
                                             start=True, stop=False)
                            for i, kb in enumerate(kbs):
                                k0 = KB * kb
                                qlo, qhi, llo, lhi = _span(kb)
                                a, b2 = max(qlo, q0), min(qhi, q0 + QSB)
                                la = a - (k0 - 2 * KB)
                                nc.tensor.matmul(po[0:65, a - q0:b2 - q0],
                                                 v_nat[:, kb, h, :],
                                                 pbt[(kb, h)][:, la:la + (b2 - a)],
                                                 start=False, stop=(i == len(kbs) - 1))
                            ob = osbp.tile([65, 512], F32, tag="ob")
                            nc.vector.tensor_copy(ob[:], po[0:65, 0:512])
                            nc.sync.dma_start(out_d[h, :, q0:q0 + 512], ob[:])

                    def do_tpose(blk):
                        pst = upsum.tile([128, 96], BF, tag="u", name=f"tp{blk}")
                        nc.tensor.transpose(pst[:], probs_g[:, 128 * blk:128 * blk + 128],
                                            id96_t[:])
                        nc.vector.tensor_copy(pb_gT[:, blk, :], pst[:])

                    emitted_kb = 0
                    emitted_qs = 0
                    for nt in range(NNT):
                        c0 = 512 * nt
                        # ---------------- group A: 128x128-mode work ----------
                        if nt == 0:
                            hst = hst0
                        else:
                            hst = hstp.tile([128, NKT, 512], BF)
                            nc.sync.dma_start(hst[:], hsT_d[:, nt, :, :])
                        for c in range(5):
                            ps = upsum.tile([128, 640], F32, tag="u")
                            for kt in range(NKT):
                                nc.tensor.matmul(ps[:, 0:512], w5_t[:, kt, c, :],
                                                 hst[:, kt, :],
                                                 start=(kt == 0), stop=(kt == NKT - 1))
                            if c == 0:
                                nc.vector.tensor_scalar_add(
                                    qT01[:, c0:c0 + 512], ps[:, 0:512], bias_t[:, 0:1])
                            elif c == 1:
                                nc.vector.tensor_scalar_add(
                                    kT01[:, c0:c0 + 512], ps[:, 0:512], bias_t[:, 1:2])
                            elif c == 2:
                                nc.vector.tensor_scalar_add(
                                    kgT01[:, c0:c0 + 512], ps[:, 0:512], bias_t[:, 2:3])
                            elif c == 3:
                                nc.scalar.activation(
                                    qT2[0:64, c0:c0 + 512], ps[0:64, 0:512], AF.Identity,
                                    bias=bias_t[0:64, 3:4], scale=1.0)
                                nc.vector.tensor_scalar_add(
                                    kgT2[64:128, c0:c0 + 512], ps[64:128, 0:512],
                                    bias_t[64:128, 3:4])
                            else:
                                nc.scalar.activation(
                                    kT2[0:64, c0:c0 + 512], ps[0:64, 0:512], AF.Identity,
                                    bias=bias_t[0:64, 4:5], scale=1.0)
                                if nt == 0:
                                    nc.vector.tensor_scalar_add(
                                        qgT2[64:128, :], ps[64:128, 0:G],
                                        bias_t[64:128, 4:5])
                        if nt == 0:
                            psq = upsum.tile([128, 640], F32, tag="u")
                            for kt in range(NKT):
                                nc.tensor.matmul(psq[:, 0:G], wqg_t[:, kt, :],
                                                 hst[:, kt, 0:G],
                                                 start=(kt == 0), stop=(kt == NKT - 1))
                            nc.vector.tensor_scalar_add(qgT01[:], psq[:, 0:G],
                                                        bias_t[:, 5:6])
                            # broadcast bvvg to all partitions via ones-matmul
                            psb = upsum.tile([128, 640], F32, tag="u")
                            nc.tensor.matmul(psb[:, 0:384], ones_t[:, 0:128], bvvg_t[:],
                                             start=True, stop=True)
                            nc.vector.tensor_copy(bvvg_b[:], psb[:, 0:384])
                        for s4 in range(4):
                            sb = 4 * nt + s4
                            psv = upsum.tile([128, 640], F32, tag="u")
                            for kt in range(NKT):
                                nc.tensor.matmul(psv[:, 0:384],
                                                 hst[:, kt, 128 * s4:128 * s4 + 128],
                                                 wvvg_t[:, kt, :],
                                                 start=(kt == 0), stop=(kt == NKT - 1))
                            nc.vector.tensor_add(
                                v_nat[:, sb, :, 0:64],
                                psv[:, 0:192].rearrange("p (h e) -> p h e", h=3),
                                bvvg_b[:, 0:192].rearrange("p (h e) -> p h e", h=3))
                            nc.vector.tensor_add(
                                vg_nat[:, sb, :, 0:64],
                                psv[:, 192:384].rearrange("p (h e) -> p h e", h=3),
                                bvvg_b[:, 192:384].rearrange("p (h e) -> p h e", h=3))
                        if nt == 0:
                            # per-head global v/ones at partitions 32h:32h+32
                            nc.vector.tensor_copy(vGp[0:32, 0, 0:64],
                                                  v_nat[0:32, 0, 0, 0:64])
                            nc.sync.dma_start(vGp[32:64, 1, 0:64],
                                              v_nat[0:32, 0, 1, 0:64])
                            nc.sync.dma_start(vGp[64:96, 2, 0:64],
                                              v_nat[0:32, 0, 2, 0:64])
                        if nt >= 1:
                            for blk in range(4 * (nt - 1), 4 * nt):
                                do_tpose(blk)
                        while emitted_qs < NQSB and 4 * emitted_qs + 5 <= emitted_kb - 1 \
                                and emitted_qs <= nt - 1:
                            do_pv(emitted_qs)
                            emitted_qs += 1
                        # ------------- group B: (64,32)-mode scores -----------
                        pssg = upsum.tile([128, 640], F32, tag="u", name=f"sg{nt}")
                        for h in range(3):
                            nc.tensor.matmul(pssg[32 * h:32 * h + 32, 0:512],
                                             sl(kT01, kT2, h)[:, 0:G],
                                             sl(qT01, qT2, h)[:, c0:c0 + 512])
                        nc.scalar.activation(exp_sg[0:96, c0:c0 + 512], pssg[0:96, 0:512], AF.Exp)
                        psgc = upsum.tile([128, 640], F32, tag="u", name=f"gc{nt}")
                        for h in range(3):
                            qg_h, kg_h = slg(h)
                            nc.tensor.matmul(psgc[32 * h:32 * h + 32, 0:512],
                                             qg_h[:], kg_h[:, c0:c0 + 512])
                        nc.scalar.activation(probs_g[:, c0:c0 + 512], psgc[0:96, 0:512], AF.Exp)
                        # ------------- group C: (64,128)-mode band ------------
                        while emitted_kb <= min(4 * nt + 1, NKB - 1):
                            do_kb(emitted_kb)
                            emitted_kb += 1
                    # tail: pv(6) overlaps the last band blocks; the global-query
                    # PV chain runs before pv(7) so its copies/DMA overlap
                    do_pv(6)
                    emitted_qs = 7
                    for blk in range(4 * (NNT - 1), NKB):
                        do_tpose(blk)
                    go = upsum.tile([128, 640], F32, tag="u", name="go")
                    og_sb = osbp.tile([96, 65], F32, tag="og")
                    for t in range(NKB):
                        for h in range(3):
                            nc.tensor.matmul(go[32 * h:32 * h + 32, 0:65],
                                             pb_gT[:, t, 32 * h:32 * h + 32],
                                             vg_nat[:, t, h, :],
                                             start=(t == 0), stop=(t == NKB - 1))
                    nc.vector.tensor_copy(og_sb[:], go[0:96, 0:65])
                    for h in range(3):
                        nc.sync.dma_start(outg_d[h], og_sb[32 * h:32 * h + 32, :])
                    while emitted_kb < NKB:
                        do_kb(emitted_kb)
                        emitted_kb += 1
                    do_pv(7)

    nc.compile()
    return nc


def _prep_inputs(inputs):
    hs = np.asarray(inputs["hidden_states"], dtype=np.float32)
    j = np.arange(KB)[None, :]
    p = np.arange(KB)[:, None]
    masks = np.stack([(j >= p), (j <= p)], axis=1).astype(bf16)  # [128, 2, 128]
    id96 = np.eye(96, dtype=bf16)

    def wtiles(w):
        n = w.shape[1]
        return np.ascontiguousarray(w.reshape(NKT, 128, n).transpose(1, 0, 2)).astype(bf16)

    maps = []
    for c in range(8):
        b, hg = c // 4, c % 4
        cols = slice(192 * hg, 192 * hg + 192)
        Wq = np.asarray(inputs["Wq"], np.float32)[:, cols] * SCALE
        bq = np.asarray(inputs["bq"], np.float32)[cols] * SCALE
        Wqg = np.asarray(inputs["Wqg"], np.float32)[:, cols] * SCALE
        bqg = np.asarray(inputs["bqg"], np.float32)[cols] * SCALE
        Wk = np.asarray(inputs["Wk"], np.float32)[:, cols]
        bk = np.asarray(inputs["bk"], np.float32)[cols]
        Wkg = np.asarray(inputs["Wkg"], np.float32)[:, cols]
        bkg = np.asarray(inputs["bkg"], np.float32)[cols]
        Wv = np.asarray(inputs["Wv"], np.float32)[:, cols]
        bv = np.asarray(inputs["bv"], np.float32)[cols]
        Wvg = np.asarray(inputs["Wvg"], np.float32)[:, cols]
        bvg = np.asarray(inputs["bvg"], np.float32)[cols]

        # 5 chains: q01, k01, kg01, [q2|kg2], [k2|qg2]
        w5 = np.concatenate([
            Wq[:, 0:128],
            Wk[:, 0:128],
            Wkg[:, 0:128],
            np.concatenate([Wq[:, 128:192], Wkg[:, 128:192]], axis=1),
            np.concatenate([Wk[:, 128:192], Wqg[:, 128:192]], axis=1),
        ], axis=1)  # [768, 640]
        w5_t = wtiles(w5).reshape(128, NKT, 5, 128)

        bias_t = np.zeros((128, 8), np.float32)
        bias_t[:, 0] = bq[0:128]
        bias_t[:, 1] = bk[0:128]
        bias_t[:, 2] = bkg[0:128]
        bias_t[0:64, 3], bias_t[64:128, 3] = bq[128:192], bkg[128:192]
        bias_t[0:64, 4], bias_t[64:128, 4] = bk[128:192], bqg[128:192]
        bias_t[:, 5] = bqg[0:128]

        hsT = np.ascontiguousarray(
            hs[b].T.reshape(NKT, 128, NNT, 512).transpose(1, 2, 0, 3)).astype(bf16)

        maps.append({
            "hsT": hsT,
            "w5": w5_t,
            "wqg01": wtiles(Wqg[:, 0:128]),
            "wvvg": wtiles(np.concatenate([Wv, Wvg], axis=1)),
            "bvvg": np.concatenate([bv, bvg])[None, :].astype(bf16),
            "bias_t": bias_t,
            "masks": masks,
            "id96": id96,
        })
    return maps


def kernel(**inputs):
    g = int(np.asarray(inputs["num_global"]))
    assert g == G, f"kernel compiled for num_global=32, got {g}"
    if "nc" not in _cache:
        _cache["nc"] = _build()
    nc = _cache["nc"]
    in_maps = _prep_inputs(inputs)
    res = run_bass_kernel_spmd(nc, in_maps, list(range(8)))
    return assemble(res.results)


def assemble(results):
    out = np.zeros((B, S, D), np.float32)
    for c in range(8):
        b, hg = c // 4, c % 4
        o = results[c]["out"]          # [3, 65, S]
        og = results[c]["outg"]        # [3, G, 65]
        for h in range(3):
            col = 192 * hg + 64 * h
            out[b, :, col:col + 64] = (o[h, 0:64] / o[h, 64]).T
            out[b, 0:G, col:col + 64] = og[h, :, 0:64] / og[h, :, 64:65]
    return out


# revision 23
# speedup vs baseline: 1.1522x; 1.1522x over previous
"""Trainium2 Bass kernel for Longformer self-attention (B=2, S=4096, D=768, H=12, HD=64, W=256, G=32).

Sharding: 8 cores = 2 batches x 4 head-groups (3 heads each). Each core computes its
batch's projections restricted to its 192 output channels, runs banded + global
attention for its 3 heads, and returns an unnormalized transposed output
([3, 65, S]: rows 0-63 = head-dim, row 64 = softmax denominator z) plus the raw
global-query output [G, 3, 65]; the host divides by z, transposes, and assembles.

Key layout/scheduling choices (v2):
  - hidden_states pre-transposed on host -> plain contiguous DMA loads.
  - 5 projection chains of full 128 output rows (q01, k01, kg01, [q2|kg2],
    [k2|qg2]) -- no half-wasted 64-row chains.
  - band scores / sg / global scores run in row-tiled PE modes; instruction
    order groups matmuls by PE tiling mode (128x128 vs 64-row modes) per
    chunk to minimize array-reconfig drains, and alternates row-tiles
    (heads 0/2 on partitions 0:64 = tile T0, head 1 on 64:128 = T8) so
    independent tiles run concurrently.
  - global-key PV contribution stays in 128-mode via zero-padded operands
    (exp_sg rows 96:128 = 0, per-head global-v at partitions 32h:32h+32).
  - band mask applied as a single bf16 multiply on the exp'd probs (SBUF),
    not f32 adds on PSUM.
  - v/vg bias via pre-broadcast bf16 add fused into the PSUM->SBUF copy.
  - PV outputs DMA'd to DRAM directly from PSUM.
Matmul inputs bf16, fp32 PSUM/softmax.
"""
import numpy as np
import ml_dtypes

import concourse.bass as bass
import concourse.mybir as mybir
import concourse.tile as tile
from concourse import bacc
from concourse.bass_utils import run_bass_kernel_spmd

B, S, D, H, HD = 2, 4096, 768, 12, 64
W = 256
G = 32
SCALE = 1.0 / np.float32(np.sqrt(HD))
KB = 128
NKB = S // KB     # 32
QSB = 512
NQSB = S // QSB   # 8
NKT = D // 128    # 6
NNT = S // 512    # 8

BF = mybir.dt.bfloat16
F32 = mybir.dt.float32
AF = mybir.ActivationFunctionType
bf16 = ml_dtypes.bfloat16

_cache = {}


def _span(kb):
    k0 = KB * kb
    qlo, qhi = max(0, k0 - 2 * KB), min(S, k0 + 3 * KB)
    return qlo, qhi, qlo - (k0 - 2 * KB), qhi - (k0 - 2 * KB)


def _build():
    nc = bacc.Bacc(None, target_bir_lowering=False)

    hsT_d = nc.declare_dram_parameter("hsT", [128, NNT, NKT, 512], BF, isOutput=False)
    w5_d = nc.declare_dram_parameter("w5", [128, NKT, 5, 128], BF, isOutput=False)
    wqg_d = nc.declare_dram_parameter("wqg01", [128, NKT, 128], BF, isOutput=False)
    wvvg_d = nc.declare_dram_parameter("wvvg", [128, NKT, 384], BF, isOutput=False)
    bvvg_d = nc.declare_dram_parameter("bvvg", [1, 384], BF, isOutput=False)
    bias_d = nc.declare_dram_parameter("bias_t", [128, 8], F32, isOutput=False)
    masks_d = nc.declare_dram_parameter("masks", [128, 2, 128], BF, isOutput=False)
    id96_d = nc.declare_dram_parameter("id96", [96, 96], BF, isOutput=False)
    out_d = nc.declare_dram_parameter("out", [3, 65, S], F32, isOutput=True)
    outg_d = nc.declare_dram_parameter("outg", [3, G, 65], F32, isOutput=True)

    with tile.TileContext(nc) as tc:
        with tc.tile_pool(name="persist", bufs=1) as pp:
            masks_t = pp.tile([128, 2, 128], BF)
            ones_t = pp.tile([1, 128], BF)
            nc.vector.memset(ones_t[:], 1.0)

            qT01 = pp.tile([128, S], BF)
            qT2 = pp.tile([128, S], BF)      # rows 0-63 used (head 2 q)
            kT01 = pp.tile([128, S], BF)
            kT2 = pp.tile([128, S], BF)      # rows 0-63 used (head 2 k)
            v_nat = pp.tile([128, NKB, 3, 65], BF)
            # exp_sg: rows 32h..32h+31 = head h's exp'd global-key scores;
            # rows 96:128 stay zero so K=128 matmuls vs vGp are exact.
            exp_sg = pp.tile([128, S], BF)
            nc.vector.memset(v_nat[:, :, :, 64:65], 1.0)
            nc.vector.memset(exp_sg[96:128, :], 0.0)

            def sl(t01, t2, h):
                return t01[64 * h:64 * h + 64] if h < 2 else t2[0:64]

            with tc.tile_pool(name="ac", bufs=1) as ac:
                kgT01 = ac.tile([128, S], BF)
                kgT2 = ac.tile([128, S], BF)  # rows 64-127 used (head 2 kg)
                qgT01 = ac.tile([128, G], BF)
                qgT2 = ac.tile([128, G], BF)  # rows 64-127 used (head 2 qg)
                vg_nat = ac.tile([128, NKB, 3, 65], BF)
                vGp = ac.tile([128, 3, 65], BF)   # head h global-v at rows 32h:32h+32
                probs_g = ac.tile([96, S], BF)
                pb_gT = ac.tile([128, NKB, 96], BF)
                id96_t = ac.tile([96, 96], BF)
                bvvg_b = ac.tile([128, 384], BF)  # bias broadcast over tokens
                nc.vector.memset(vg_nat[:, :, :, 64:65], 1.0)
                nc.vector.memset(vGp[:], 0.0)
                for h in range(3):
                    nc.vector.memset(vGp[32 * h:32 * h + 32, h, 64:65], 1.0)

                def slg(h):
                    if h < 2:
                        return qgT01[64 * h:64 * h + 64], kgT01[64 * h:64 * h + 64]
                    return qgT2[64:128], kgT2[64:128]

                with (
                    tc.tile_pool(name="aw", bufs=1) as aw,
                    tc.tile_pool(name="hst", bufs=3) as hstp,
                    tc.tile_pool(name="apsum", bufs=2, space="PSUM") as apsum,
                    tc.tile_pool(name="spsum", bufs=3, space="PSUM") as spsum,
                    tc.tile_pool(name="pbt", bufs=30) as pbtp,
                    tc.tile_pool(name="osb", bufs=3) as osbp,
                ):
                    w5_t = aw.tile([128, NKT, 5, 128], BF)
                    wqg_t = aw.tile([128, NKT, 128], BF)
                    wvvg_t = aw.tile([128, NKT, 384], BF)
                    bvvg_t = aw.tile([1, 384], BF)
                    bias_t = aw.tile([128, 8], F32)
                    # issue startup DMAs from several engines in parallel;
                    # w5 + first hs chunk are on the critical path
                    nc.sync.dma_start(w5_t[:], w5_d[:])
                    hst0 = hstp.tile([128, NKT, 512], BF)
                    nc.sync.dma_start(hst0[:], hsT_d[:, 0, :, :])
                    nc.scalar.dma_start(wvvg_t[:], wvvg_d[:])
                    nc.scalar.dma_start(bias_t[:], bias_d[:])
                    nc.gpsimd.dma_start(wqg_t[:], wqg_d[:])
                    nc.gpsimd.dma_start(bvvg_t[:], bvvg_d[:])
                    nc.gpsimd.dma_start(masks_t[:], masks_d[:])
                    nc.gpsimd.dma_start(id96_t[:], id96_d[:])

                    pbt = {}

                    def mm_score(t, h, kb, a, b2):
                        k0 = KB * kb
                        qlo, qhi, llo, lhi = _span(kb)
                        nc.tensor.matmul(
                            t[:, a:b2],
                            sl(kT01, kT2, h)[:, k0:k0 + KB],
                            sl(qT01, qT2, h)[:, qlo + (a - llo):qlo + (a - llo) + (b2 - a)])

                    def exp_mask(ps, kb, h):
                        qlo, qhi, llo, lhi = _span(kb)
                        t_ = pbtp.tile([128, 640], BF, tag="pb")
                        nc.scalar.activation(t_[:, llo:lhi], ps[:, llo:lhi], AF.Exp)
                        # zero the out-of-window triangles (cols 0:128 and
                        # 512:640) with one strided bf16 multiply on GpSimd
                        tv = t_.rearrange("p (o j) -> p o j", o=5)
                        if llo == 0 and lhi == 640:
                            nc.vector.tensor_mul(tv[:, 0:5:4, :], tv[:, 0:5:4, :],
                                                 masks_t[:])
                        elif llo == 0:
                            nc.vector.tensor_mul(tv[:, 0, :], tv[:, 0, :],
                                                 masks_t[:, 0, :])
                        else:
                            nc.vector.tensor_mul(tv[:, 4, :], tv[:, 4, :],
                                                 masks_t[:, 1, :])
                        pbt[(kb, h)] = t_

                    def do_kb(kb):
                        qlo, qhi, llo, lhi = _span(kb)
                        pieces = [(a, b) for (a, b) in
                                  [(llo, min(lhi, 512)), (max(llo, 512), lhi)] if a < b]
                        tiles = []
                        for h in range(3):
                            tiles.append(spsum.tile([128, 640], F32, tag="sc",
                                                    name=f"sc{kb}_{h}"))
                        # h0 on PE row-tile T0, h1 on T8: interleave their pieces
                        # so the two tiles run concurrently; h2 (T0) goes last so
                        # its PSUM slot (shared with h0) is free by then.
                        first = [0, 1] if kb % 2 == 0 else [1, 0]
                        for (a, b2) in pieces:
                            for h in first:
                                mm_score(tiles[h], h, kb, a, b2)
                        for h in first:
                            exp_mask(tiles[h], kb, h)
                        for (a, b2) in pieces:
                            mm_score(tiles[2], 2, kb, a, b2)
                        exp_mask(tiles[2], kb, 2)

                    def do_pv(qs):
                        q0 = QSB * qs
                        kbs = list(range(max(0, 4 * qs - 2), min(NKB, 4 * qs + 6)))
                        for h in range(3):
                            po = apsum.tile([128, 512], F32, tag="pp")
                            nc.tensor.matmul(po[0:65, 0:512], vGp[:, h, :],
                                             exp_sg[:, q0:q0 + 512],
                                             start=True, stop=False)
                            for i, kb in enumerate(kbs):
                                k0 = KB * kb
                                qlo, qhi, llo, lhi = _span(kb)
                                a, b2 = max(qlo, q0), min(qhi, q0 + QSB)
                                la = a - (k0 - 2 * KB)
                                nc.tensor.matmul(po[0:65, a - q0:b2 - q0],
                                                 v_nat[:, kb, h, :],
                                                 pbt[(kb, h)][:, la:la + (b2 - a)],
                                                 start=False, stop=(i == len(kbs) - 1))
                            ob = osbp.tile([65, 512], F32, tag="ob")
                            nc.vector.tensor_copy(ob[:], po[0:65, 0:512])
                            nc.sync.dma_start(out_d[h, :, q0:q0 + 512], ob[:])

                    def do_tpose(blk):
                        pst = apsum.tile([128, 96], BF, tag="pp", name=f"tp{blk}")
                        nc.tensor.transpose(pst[:], probs_g[:, 128 * blk:128 * blk + 128],
                                            id96_t[:])
                        nc.vector.tensor_copy(pb_gT[:, blk, :], pst[:])

                    emitted_kb = 0
                    emitted_qs = 0
                    for nt in range(NNT):
                        c0 = 512 * nt
                        # ---------------- group A: 128x128-mode work ----------
                        if nt == 0:
                            hst = hst0
                        else:
                            hst = hstp.tile([128, NKT, 512], BF)
                            nc.sync.dma_start(hst[:], hsT_d[:, nt, :, :])
                        for c in range(5):
                            ps = apsum.tile([128, 512], F32, tag="pp")
                            for kt in range(NKT):
                                nc.tensor.matmul(ps[:, 0:512], w5_t[:, kt, c, :],
                                                 hst[:, kt, :],
                                                 start=(kt == 0), stop=(kt == NKT - 1))
                            if c == 0:
                                nc.vector.tensor_scalar_add(
                                    qT01[:, c0:c0 + 512], ps[:, 0:512], bias_t[:, 0:1])
                            elif c == 1:
                                nc.vector.tensor_scalar_add(
                                    kT01[:, c0:c0 + 512], ps[:, 0:512], bias_t[:, 1:2])
                            elif c == 2:
                                nc.vector.tensor_scalar_add(
                                    kgT01[:, c0:c0 + 512], ps[:, 0:512], bias_t[:, 2:3])
                            elif c == 3:
                                nc.scalar.activation(
                                    qT2[0:64, c0:c0 + 512], ps[0:64, 0:512], AF.Identity,
                                    bias=bias_t[0:64, 3:4], scale=1.0)
                                nc.vector.tensor_scalar_add(
                                    kgT2[64:128, c0:c0 + 512], ps[64:128, 0:512],
                                    bias_t[64:128, 3:4])
                            else:
                                nc.scalar.activation(
                                    kT2[0:64, c0:c0 + 512], ps[0:64, 0:512], AF.Identity,
                                    bias=bias_t[0:64, 4:5], scale=1.0)
                                if nt == 0:
                                    nc.vector.tensor_scalar_add(
                                        qgT2[64:128, :], ps[64:128, 0:G],
                                        bias_t[64:128, 4:5])
                        if nt == 0:
                            psq = apsum.tile([128, 512], F32, tag="pp")
                            for kt in range(NKT):
                                nc.tensor.matmul(psq[:, 0:G], wqg_t[:, kt, :],
                                                 hst[:, kt, 0:G],
                                                 start=(kt == 0), stop=(kt == NKT - 1))
                            nc.vector.tensor_scalar_add(qgT01[:], psq[:, 0:G],
                                                        bias_t[:, 5:6])
                            # broadcast bvvg to all partitions via ones-matmul
                            psb = apsum.tile([128, 512], F32, tag="pp")
                            nc.tensor.matmul(psb[:, 0:384], ones_t[:, 0:128], bvvg_t[:],
                                             start=True, stop=True)
                            nc.vector.tensor_copy(bvvg_b[:], psb[:, 0:384])
                        for s4 in range(4):
                            sb = 4 * nt + s4
                            psv = apsum.tile([128, 512], F32, tag="pp")
                            for kt in range(NKT):
                                nc.tensor.matmul(psv[:, 0:384],
                                                 hst[:, kt, 128 * s4:128 * s4 + 128],
                                                 wvvg_t[:, kt, :],
                                                 start=(kt == 0), stop=(kt == NKT - 1))
                            nc.vector.tensor_add(
                                v_nat[:, sb, :, 0:64],
                                psv[:, 0:192].rearrange("p (h e) -> p h e", h=3),
                                bvvg_b[:, 0:192].rearrange("p (h e) -> p h e", h=3))
                            nc.vector.tensor_add(
                                vg_nat[:, sb, :, 0:64],
                                psv[:, 192:384].rearrange("p (h e) -> p h e", h=3),
                                bvvg_b[:, 192:384].rearrange("p (h e) -> p h e", h=3))
                        if nt == 0:
                            # per-head global v/ones at partitions 32h:32h+32
                            nc.vector.tensor_copy(vGp[0:32, 0, 0:64],
                                                  v_nat[0:32, 0, 0, 0:64])
                            nc.sync.dma_start(vGp[32:64, 1, 0:64],
                                              v_nat[0:32, 0, 1, 0:64])
                            nc.sync.dma_start(vGp[64:96, 2, 0:64],
                                              v_nat[0:32, 0, 2, 0:64])
                        if nt >= 1:
                            for blk in range(4 * (nt - 1), 4 * nt):
                                do_tpose(blk)
                        while emitted_qs < NQSB and 4 * emitted_qs + 5 <= emitted_kb - 1 \
                                and emitted_qs <= nt - 1:
                            do_pv(emitted_qs)
                            emitted_qs += 1
                        # ------------- group B: (64,32)-mode scores -----------
                        pssg = apsum.tile([128, 512], F32, tag="pp", name=f"sg{nt}")
                        for h in range(3):
                            nc.tensor.matmul(pssg[32 * h:32 * h + 32, 0:512],
                                             sl(kT01, kT2, h)[:, 0:G],
                                             sl(qT01, qT2, h)[:, c0:c0 + 512])
                        nc.scalar.activation(exp_sg[0:96, c0:c0 + 512], pssg[0:96, 0:512], AF.Exp)
                        psgc = apsum.tile([128, 512], F32, tag="pp", name=f"gc{nt}")
                        for h in range(3):
                            qg_h, kg_h = slg(h)
                            nc.tensor.matmul(psgc[32 * h:32 * h + 32, 0:512],
                                             qg_h[:], kg_h[:, c0:c0 + 512])
                        nc.scalar.activation(probs_g[:, c0:c0 + 512], psgc[0:96, 0:512], AF.Exp)
                        # ------------- group C: (64,128)-mode band ------------
                        while emitted_kb <= min(4 * nt + 1, NKB - 1):
                            do_kb(emitted_kb)
                            emitted_kb += 1
                    # tail: pv(6) overlaps the last band blocks; the global-query
                    # PV chain runs before pv(7) so its copies/DMA overlap
                    do_pv(6)
                    emitted_qs = 7
                    for blk in range(4 * (NNT - 1), NKB):
                        do_tpose(blk)
                    go = apsum.tile([128, 512], F32, tag="pp", name="go")
                    og_sb = osbp.tile([96, 65], F32, tag="og")
                    for t in range(NKB):
                        for h in range(3):
                            nc.tensor.matmul(go[32 * h:32 * h + 32, 0:65],
                                             pb_gT[:, t, 32 * h:32 * h + 32],
                                             vg_nat[:, t, h, :],
                                             start=(t == 0), stop=(t == NKB - 1))
                    nc.vector.tensor_copy(og_sb[:], go[0:96, 0:65])
                    for h in range(3):
                        nc.sync.dma_start(outg_d[h], og_sb[32 * h:32 * h + 32, :])
                    while emitted_kb < NKB:
                        do_kb(emitted_kb)
                        emitted_kb += 1
                    do_pv(7)

    nc.compile()
    return nc


def _prep_inputs(inputs):
    hs = np.asarray(inputs["hidden_states"], dtype=np.float32)
    j = np.arange(KB)[None, :]
    p = np.arange(KB)[:, None]
    masks = np.stack([(j >= p), (j <= p)], axis=1).astype(bf16)  # [128, 2, 128]
    id96 = np.eye(96, dtype=bf16)

    def wtiles(w):
        n = w.shape[1]
        return np.ascontiguousarray(w.reshape(NKT, 128, n).transpose(1, 0, 2)).astype(bf16)

    maps = []
    for c in range(8):
        b, hg = c // 4, c % 4
        cols = slice(192 * hg, 192 * hg + 192)
        Wq = np.asarray(inputs["Wq"], np.float32)[:, cols] * SCALE
        bq = np.asarray(inputs["bq"], np.float32)[cols] * SCALE
        Wqg = np.asarray(inputs["Wqg"], np.float32)[:, cols] * SCALE
        bqg = np.asarray(inputs["bqg"], np.float32)[cols] * SCALE
        Wk = np.asarray(inputs["Wk"], np.float32)[:, cols]
        bk = np.asarray(inputs["bk"], np.float32)[cols]
        Wkg = np.asarray(inputs["Wkg"], np.float32)[:, cols]
        bkg = np.asarray(inputs["bkg"], np.float32)[cols]
        Wv = np.asarray(inputs["Wv"], np.float32)[:, cols]
        bv = np.asarray(inputs["bv"], np.float32)[cols]
        Wvg = np.asarray(inputs["Wvg"], np.float32)[:, cols]
        bvg = np.asarray(inputs["bvg"], np.float32)[cols]

        # 5 chains: q01, k01, kg01, [q2|kg2], [k2|qg2]
        w5 = np.concatenate([
            Wq[:, 0:128],
            Wk[:, 0:128],
            Wkg[:, 0:128],
            np.concatenate([Wq[:, 128:192], Wkg[:, 128:192]], axis=1),
            np.concatenate([Wk[:, 128:192], Wqg[:, 128:192]], axis=1),
        ], axis=1)  # [768, 640]
        w5_t = wtiles(w5).reshape(128, NKT, 5, 128)

        bias_t = np.zeros((128, 8), np.float32)
        bias_t[:, 0] = bq[0:128]
        bias_t[:, 1] = bk[0:128]
        bias_t[:, 2] = bkg[0:128]
        bias_t[0:64, 3], bias_t[64:128, 3] = bq[128:192], bkg[128:192]
        bias_t[0:64, 4], bias_t[64:128, 4] = bk[128:192], bqg[128:192]
        bias_t[:, 5] = bqg[0:128]

        hsT = np.ascontiguousarray(
            hs[b].T.reshape(NKT, 128, NNT, 512).transpose(1, 2, 0, 3)).astype(bf16)

        maps.append({
            "hsT": hsT,
            "w5": w5_t,
            "wqg01": wtiles(Wqg[:, 0:128]),
            "wvvg": wtiles(np.concatenate([Wv, Wvg], axis=1)),
            "bvvg": np.concatenate([bv, bvg])[None, :].astype(bf16),
            "bias_t": bias_t,
            "masks": masks,
            "id96": id96,
        })
    return maps


def kernel(**inputs):
    g = int(np.asarray(inputs["num_global"]))
    assert g == G, f"kernel compiled for num_global=32, got {g}"
    if "nc" not in _cache:
        _cache["nc"] = _build()
    nc = _cache["nc"]
    in_maps = _prep_inputs(inputs)
    res = run_bass_kernel_spmd(nc, in_maps, list(range(8)))
    return assemble(res.results)


def assemble(results):
    out = np.zeros((B, S, D), np.float32)
    for c in range(8):
        b, hg = c // 4, c % 4
        o = results[c]["out"]          # [3, 65, S]
        og = results[c]["outg"]        # [3, G, 65]
        for h in range(3):
            col = 192 * hg + 64 * h
            out[b, :, col:col + 64] = (o[h, 0:64] / o[h, 64]).T
            out[b, 0:G, col:col + 64] = og[h, :, 0:64] / og[h, :, 64:65]
    return out


# revision 25
# speedup vs baseline: 1.3822x; 1.1996x over previous
"""Trainium2 Bass kernel for Longformer self-attention (B=2, S=4096, D=768, H=12, HD=64, W=256, G=32).

Sharding: 8 cores = 2 batches x 4 head-groups (3 heads each). Each core computes its
batch's projections restricted to its 192 output channels, runs banded + global
attention for its 3 heads, and returns an unnormalized transposed output
([3, 65, S]: rows 0-63 = head-dim, row 64 = softmax denominator z) plus the raw
global-query output [G, 3, 65]; the host divides by z, transposes, and assembles.

Key layout/scheduling choices (v2):
  - hidden_states pre-transposed on host -> plain contiguous DMA loads.
  - 5 projection chains of full 128 output rows (q01, k01, kg01, [q2|kg2],
    [k2|qg2]) -- no half-wasted 64-row chains.
  - band scores / sg / global scores run in row-tiled PE modes; instruction
    order groups matmuls by PE tiling mode (128x128 vs 64-row modes) per
    chunk to minimize array-reconfig drains, and alternates row-tiles
    (heads 0/2 on partitions 0:64 = tile T0, head 1 on 64:128 = T8) so
    independent tiles run concurrently.
  - global-key PV contribution stays in 128-mode via zero-padded operands
    (exp_sg rows 96:128 = 0, per-head global-v at partitions 32h:32h+32).
  - band mask applied as a single bf16 multiply on the exp'd probs (SBUF),
    not f32 adds on PSUM.
  - v/vg bias via pre-broadcast bf16 add fused into the PSUM->SBUF copy.
  - PV outputs DMA'd to DRAM directly from PSUM.
Matmul inputs bf16, fp32 PSUM/softmax.
"""
import numpy as np
import ml_dtypes

import concourse.bass as bass
import concourse.mybir as mybir
import concourse.tile as tile
from concourse import bacc
from concourse.bass_utils import run_bass_kernel_spmd

B, S, D, H, HD = 2, 4096, 768, 12, 64
W = 256
G = 32
SCALE = 1.0 / np.float32(np.sqrt(HD))
KB = 128
NKB = S // KB     # 32
QSB = 512
NQSB = S // QSB   # 8
NKT = D // 128    # 6
NNT = S // 512    # 8

BF = mybir.dt.bfloat16
F32 = mybir.dt.float32
AF = mybir.ActivationFunctionType
bf16 = ml_dtypes.bfloat16

_cache = {}


def _span(kb):
    k0 = KB * kb
    qlo, qhi = max(0, k0 - 2 * KB), min(S, k0 + 3 * KB)
    return qlo, qhi, qlo - (k0 - 2 * KB), qhi - (k0 - 2 * KB)


def _build():
    nc = bacc.Bacc(None, target_bir_lowering=False)

    hsT_d = nc.declare_dram_parameter("hsT", [128, NNT, NKT, 512], BF, isOutput=False)
    w5_d = nc.declare_dram_parameter("w5", [128, NKT, 5, 128], BF, isOutput=False)
    wqg_d = nc.declare_dram_parameter("wqg01", [128, NKT, 128], BF, isOutput=False)
    wvvg_d = nc.declare_dram_parameter("wvvg", [128, NKT, 384], BF, isOutput=False)
    bvvg_d = nc.declare_dram_parameter("bvvg", [1, 384], BF, isOutput=False)
    bias_d = nc.declare_dram_parameter("bias_t", [128, 8], F32, isOutput=False)
    masks_d = nc.declare_dram_parameter("masks", [128, 2, 128], BF, isOutput=False)
    id96_d = nc.declare_dram_parameter("id96", [96, 96], BF, isOutput=False)
    out_d = nc.declare_dram_parameter("out", [3, 65, S], F32, isOutput=True)
    outg_d = nc.declare_dram_parameter("outg", [3, G, 65], F32, isOutput=True)

    with tile.TileContext(nc) as tc:
        with tc.tile_pool(name="persist", bufs=1) as pp:
            masks_t = pp.tile([128, 2, 128], BF)
            ones_t = pp.tile([1, 128], BF)
            nc.gpsimd.memset(ones_t[:], 1.0)

            qT01 = pp.tile([128, S], BF)
            qT2 = pp.tile([128, S], BF)      # rows 0-63 used (head 2 q)
            kT01 = pp.tile([128, S], BF)
            kT2 = pp.tile([128, S], BF)      # rows 0-63 used (head 2 k)
            v_nat = pp.tile([128, NKB, 3, 65], BF)
            # exp_sg: rows 32h..32h+31 = head h's exp'd global-key scores;
            # rows 96:128 stay zero so K=128 matmuls vs vGp are exact.
            exp_sg = pp.tile([128, S], BF)
            nc.gpsimd.memset(v_nat[:, :, :, 64:65], 1.0)
            nc.gpsimd.memset(exp_sg[96:128, :], 0.0)

            def sl(t01, t2, h):
                return t01[64 * h:64 * h + 64] if h < 2 else t2[0:64]

            with tc.tile_pool(name="ac", bufs=1) as ac:
                kgT01 = ac.tile([128, S], BF)
                kgT2 = ac.tile([128, S], BF)  # rows 64-127 used (head 2 kg)
                qgT01 = ac.tile([128, G], BF)
                qgT2 = ac.tile([128, G], BF)  # rows 64-127 used (head 2 qg)
                vg_nat = ac.tile([128, NKB, 3, 65], BF)
                vGp = ac.tile([128, 3, 65], BF)   # head h global-v at rows 32h:32h+32
                probs_g = ac.tile([96, S], BF)
                pb_gT = ac.tile([128, NKB, 96], BF)
                id96_t = ac.tile([96, 96], BF)
                bvvg_b = ac.tile([128, 384], BF)  # bias broadcast over tokens
                nc.gpsimd.memset(vg_nat[:, :, :, 64:65], 1.0)
                nc.gpsimd.memset(vGp[:], 0.0)
                for h in range(3):
                    nc.gpsimd.memset(vGp[32 * h:32 * h + 32, h, 64:65], 1.0)

                def slg(h):
                    if h < 2:
                        return qgT01[64 * h:64 * h + 64], kgT01[64 * h:64 * h + 64]
                    return qgT2[64:128], kgT2[64:128]

                with (
                    tc.tile_pool(name="aw", bufs=1) as aw,
                    tc.tile_pool(name="hst", bufs=3) as hstp,
                    tc.tile_pool(name="apsum", bufs=2, space="PSUM") as apsum,
                    tc.tile_pool(name="spsum", bufs=2, space="PSUM") as spsum,
                    tc.tile_pool(name="opsum", bufs=2, space="PSUM") as opsum,
                    tc.tile_pool(name="pbt", bufs=30) as pbtp,
                    tc.tile_pool(name="osb", bufs=3) as osbp,
                ):
                    w5_t = aw.tile([128, NKT, 5, 128], BF)
                    wqg_t = aw.tile([128, NKT, 128], BF)
                    wvvg_t = aw.tile([128, NKT, 384], BF)
                    bvvg_t = aw.tile([1, 384], BF)
                    bias_t = aw.tile([128, 8], F32)
                    # issue startup DMAs from several engines in parallel;
                    # w5 + first hs chunk are on the critical path
                    nc.sync.dma_start(w5_t[:], w5_d[:])
                    hst0 = hstp.tile([128, NKT, 512], BF)
                    nc.sync.dma_start(hst0[:], hsT_d[:, 0, :, :])
                    nc.scalar.dma_start(wvvg_t[:], wvvg_d[:])
                    nc.scalar.dma_start(bias_t[:], bias_d[:])
                    nc.gpsimd.dma_start(wqg_t[:], wqg_d[:])
                    nc.gpsimd.dma_start(bvvg_t[:], bvvg_d[:])
                    nc.gpsimd.dma_start(masks_t[:], masks_d[:])
                    nc.gpsimd.dma_start(id96_t[:], id96_d[:])

                    pbt = {}

                    def mm_score(t, h, kb, a, b2):
                        k0 = KB * kb
                        qlo, qhi, llo, lhi = _span(kb)
                        nc.tensor.matmul(
                            t[:, a:b2],
                            sl(kT01, kT2, h)[:, k0:k0 + KB],
                            sl(qT01, qT2, h)[:, qlo + (a - llo):qlo + (a - llo) + (b2 - a)])

                    def exp_mask(ps, kb, h):
                        qlo, qhi, llo, lhi = _span(kb)
                        t_ = pbtp.tile([128, 640], BF, tag="pb")
                        nc.scalar.activation(t_[:, llo:lhi], ps[:, llo:lhi], AF.Exp)
                        # zero the out-of-window triangles (cols 0:128 and
                        # 512:640) with one strided bf16 multiply on GpSimd
                        tv = t_.rearrange("p (o j) -> p o j", o=5)
                        if llo == 0 and lhi == 640:
                            nc.vector.tensor_mul(tv[:, 0:5:4, :], tv[:, 0:5:4, :],
                                                 masks_t[:])
                        elif llo == 0:
                            nc.vector.tensor_mul(tv[:, 0, :], tv[:, 0, :],
                                                 masks_t[:, 0, :])
                        else:
                            nc.vector.tensor_mul(tv[:, 4, :], tv[:, 4, :],
                                                 masks_t[:, 1, :])
                        pbt[(kb, h)] = t_

                    def do_kb(kb):
                        qlo, qhi, llo, lhi = _span(kb)
                        pieces = [(a, b) for (a, b) in
                                  [(llo, min(lhi, 512)), (max(llo, 512), lhi)] if a < b]
                        tiles = []
                        for h in range(3):
                            tiles.append(spsum.tile([128, 640], F32, tag="sc",
                                                    name=f"sc{kb}_{h}"))
                        # h0 on PE row-tile T0, h1 on T8: interleave their pieces
                        # so the two tiles run concurrently; h2 (T0) goes last so
                        # its PSUM slot (shared with h0) is free by then.
                        first = [0, 1] if kb % 2 == 0 else [1, 0]
                        for (a, b2) in pieces:
                            for h in first:
                                mm_score(tiles[h], h, kb, a, b2)
                        for h in first:
                            exp_mask(tiles[h], kb, h)
                        for (a, b2) in pieces:
                            mm_score(tiles[2], 2, kb, a, b2)
                        exp_mask(tiles[2], kb, 2)

                    def do_pv(qs):
                        q0 = QSB * qs
                        kbs = list(range(max(0, 4 * qs - 2), min(NKB, 4 * qs + 6)))
                        for h in range(3):
                            po = opsum.tile([96, 512], F32, tag="po")
                            nc.tensor.matmul(po[0:65, :], vGp[:, h, :],
                                             exp_sg[:, q0:q0 + 512],
                                             start=True, stop=False)
                            for i, kb in enumerate(kbs):
                                k0 = KB * kb
                                qlo, qhi, llo, lhi = _span(kb)
                                a, b2 = max(qlo, q0), min(qhi, q0 + QSB)
                                la = a - (k0 - 2 * KB)
                                nc.tensor.matmul(po[0:65, a - q0:b2 - q0],
                                                 v_nat[:, kb, h, :],
                                                 pbt[(kb, h)][:, la:la + (b2 - a)],
                                                 start=False, stop=(i == len(kbs) - 1))
                            ob = osbp.tile([65, 512], F32, tag="ob")
                            nc.vector.tensor_copy(ob[:], po[0:65, :])
                            nc.sync.dma_start(out_d[h, :, q0:q0 + 512], ob[:])

                    def do_tpose(blk):
                        pst = apsum.tile([128, 96], BF, tag="pp", name=f"tp{blk}")
                        nc.tensor.transpose(pst[:], probs_g[:, 128 * blk:128 * blk + 128],
                                            id96_t[:])
                        nc.vector.tensor_copy(pb_gT[:, blk, :], pst[:])

                    emitted_kb = 0
                    emitted_qs = 0
                    for nt in range(NNT):
                        c0 = 512 * nt
                        # ---------------- group A: 128x128-mode work ----------
                        if nt == 0:
                            hst = hst0
                        else:
                            hst = hstp.tile([128, NKT, 512], BF)
                            nc.sync.dma_start(hst[:], hsT_d[:, nt, :, :])
                        for c in range(5):
                            ps = apsum.tile([128, 512], F32, tag="pp")
                            for kt in range(NKT):
                                nc.tensor.matmul(ps[:, 0:512], w5_t[:, kt, c, :],
                                                 hst[:, kt, :],
                                                 start=(kt == 0), stop=(kt == NKT - 1))
                            if c == 0:
                                nc.vector.tensor_scalar_add(
                                    qT01[:, c0:c0 + 512], ps[:, 0:512], bias_t[:, 0:1])
                            elif c == 1:
                                nc.vector.tensor_scalar_add(
                                    kT01[:, c0:c0 + 512], ps[:, 0:512], bias_t[:, 1:2])
                            elif c == 2:
                                nc.vector.tensor_scalar_add(
                                    kgT01[:, c0:c0 + 512], ps[:, 0:512], bias_t[:, 2:3])
                            elif c == 3:
                                nc.vector.tensor_scalar_add(
                                    qT2[0:64, c0:c0 + 512], ps[0:64, 0:512],
                                    bias_t[0:64, 3:4])
                                nc.vector.tensor_scalar_add(
                                    kgT2[64:128, c0:c0 + 512], ps[64:128, 0:512],
                                    bias_t[64:128, 3:4])
                            else:
                                nc.vector.tensor_scalar_add(
                                    kT2[0:64, c0:c0 + 512], ps[0:64, 0:512],
                                    bias_t[0:64, 4:5])
                                if nt == 0:
                                    nc.vector.tensor_scalar_add(
                                        qgT2[64:128, :], ps[64:128, 0:G],
                                        bias_t[64:128, 4:5])
                        if nt == 0:
                            psq = apsum.tile([128, 512], F32, tag="pp")
                            for kt in range(NKT):
                                nc.tensor.matmul(psq[:, 0:G], wqg_t[:, kt, :],
                                                 hst[:, kt, 0:G],
                                                 start=(kt == 0), stop=(kt == NKT - 1))
                            nc.vector.tensor_scalar_add(qgT01[:], psq[:, 0:G],
                                                        bias_t[:, 5:6])
                            # broadcast bvvg to all partitions via ones-matmul
                            psb = apsum.tile([128, 512], F32, tag="pp")
                            nc.tensor.matmul(psb[:, 0:384], ones_t[:, 0:128], bvvg_t[:],
                                             start=True, stop=True)
                            nc.vector.tensor_copy(bvvg_b[:], psb[:, 0:384])
                        for s4 in range(4):
                            sb = 4 * nt + s4
                            psv = apsum.tile([128, 512], F32, tag="pp")
                            for kt in range(NKT):
                                nc.tensor.matmul(psv[:, 0:384],
                                                 hst[:, kt, 128 * s4:128 * s4 + 128],
                                                 wvvg_t[:, kt, :],
                                                 start=(kt == 0), stop=(kt == NKT - 1))
                            nc.vector.tensor_add(
                                v_nat[:, sb, :, 0:64],
                                psv[:, 0:192].rearrange("p (h e) -> p h e", h=3),
                                bvvg_b[:, 0:192].rearrange("p (h e) -> p h e", h=3))
                            nc.vector.tensor_add(
                                vg_nat[:, sb, :, 0:64],
                                psv[:, 192:384].rearrange("p (h e) -> p h e", h=3),
                                bvvg_b[:, 192:384].rearrange("p (h e) -> p h e", h=3))
                        if nt == 0:
                            # per-head global v/ones at partitions 32h:32h+32
                            nc.vector.tensor_copy(vGp[0:32, 0, 0:64],
                                                  v_nat[0:32, 0, 0, 0:64])
                            nc.sync.dma_start(vGp[32:64, 1, 0:64],
                                              v_nat[0:32, 0, 1, 0:64])
                            nc.sync.dma_start(vGp[64:96, 2, 0:64],
                                              v_nat[0:32, 0, 2, 0:64])
                        if nt >= 1:
                            for blk in range(4 * (nt - 1), 4 * nt):
                                do_tpose(blk)
                        while emitted_qs < NQSB and 4 * emitted_qs + 5 <= emitted_kb - 1 \
                                and emitted_qs <= nt - 1:
                            do_pv(emitted_qs)
                            emitted_qs += 1
                        # ------------- group B: (64,32)-mode scores -----------
                        pssg = opsum.tile([96, 512], F32, tag="po", name=f"sg{nt}")
                        for h in range(3):
                            nc.tensor.matmul(pssg[32 * h:32 * h + 32, :],
                                             sl(kT01, kT2, h)[:, 0:G],
                                             sl(qT01, qT2, h)[:, c0:c0 + 512])
                        nc.scalar.activation(exp_sg[0:96, c0:c0 + 512], pssg[:], AF.Exp)
                        psgc = opsum.tile([96, 512], F32, tag="po", name=f"gc{nt}")
                        for h in range(3):
                            qg_h, kg_h = slg(h)
                            nc.tensor.matmul(psgc[32 * h:32 * h + 32, :],
                                             qg_h[:], kg_h[:, c0:c0 + 512])
                        nc.scalar.activation(probs_g[:, c0:c0 + 512], psgc[:], AF.Exp)
                        # ------------- group C: (64,128)-mode band ------------
                        while emitted_kb <= min(4 * nt + 1, NKB - 1):
                            do_kb(emitted_kb)
                            emitted_kb += 1
                    # tail: pv(6) overlaps the last band blocks; the global-query
                    # PV chain runs before pv(7) so its copies/DMA overlap
                    do_pv(6)
                    emitted_qs = 7
                    for blk in range(4 * (NNT - 1), NKB):
                        do_tpose(blk)
                    go = apsum.tile([128, 512], F32, tag="pp", name="go")
                    og_sb = osbp.tile([96, 65], F32, tag="og")
                    for t in range(NKB):
                        for h in range(3):
                            nc.tensor.matmul(go[32 * h:32 * h + 32, 0:65],
                                             pb_gT[:, t, 32 * h:32 * h + 32],
                                             vg_nat[:, t, h, :],
                                             start=(t == 0), stop=(t == NKB - 1))
                    nc.vector.tensor_copy(og_sb[:], go[0:96, 0:65])
                    for h in range(3):
                        nc.sync.dma_start(outg_d[h], og_sb[32 * h:32 * h + 32, :])
                    while emitted_kb < NKB:
                        do_kb(emitted_kb)
                        emitted_kb += 1
                    do_pv(7)

    nc.compile()
    return nc


def _prep_inputs(inputs):
    hs = np.asarray(inputs["hidden_states"], dtype=np.float32)
    j = np.arange(KB)[None, :]
    p = np.arange(KB)[:, None]
    masks = np.stack([(j >= p), (j <= p)], axis=1).astype(bf16)  # [128, 2, 128]
    id96 = np.eye(96, dtype=bf16)

    def wtiles(w):
        n = w.shape[1]
        return np.ascontiguousarray(w.reshape(NKT, 128, n).transpose(1, 0, 2)).astype(bf16)

    maps = []
    for c in range(8):
        b, hg = c // 4, c % 4
        cols = slice(192 * hg, 192 * hg + 192)
        Wq = np.asarray(inputs["Wq"], np.float32)[:, cols] * SCALE
        bq = np.asarray(inputs["bq"], np.float32)[cols] * SCALE
        Wqg = np.asarray(inputs["Wqg"], np.float32)[:, cols] * SCALE
        bqg = np.asarray(inputs["bqg"], np.float32)[cols] * SCALE
        Wk = np.asarray(inputs["Wk"], np.float32)[:, cols]
        bk = np.asarray(inputs["bk"], np.float32)[cols]
        Wkg = np.asarray(inputs["Wkg"], np.float32)[:, cols]
        bkg = np.asarray(inputs["bkg"], np.float32)[cols]
        Wv = np.asarray(inputs["Wv"], np.float32)[:, cols]
        bv = np.asarray(inputs["bv"], np.float32)[cols]
        Wvg = np.asarray(inputs["Wvg"], np.float32)[:, cols]
        bvg = np.asarray(inputs["bvg"], np.float32)[cols]

        # 5 chains: q01, k01, kg01, [q2|kg2], [k2|qg2]
        w5 = np.concatenate([
            Wq[:, 0:128],
            Wk[:, 0:128],
            Wkg[:, 0:128],
            np.concatenate([Wq[:, 128:192], Wkg[:, 128:192]], axis=1),
            np.concatenate([Wk[:, 128:192], Wqg[:, 128:192]], axis=1),
        ], axis=1)  # [768, 640]
        w5_t = wtiles(w5).reshape(128, NKT, 5, 128)

        bias_t = np.zeros((128, 8), np.float32)
        bias_t[:, 0] = bq[0:128]
        bias_t[:, 1] = bk[0:128]
        bias_t[:, 2] = bkg[0:128]
        bias_t[0:64, 3], bias_t[64:128, 3] = bq[128:192], bkg[128:192]
        bias_t[0:64, 4], bias_t[64:128, 4] = bk[128:192], bqg[128:192]
        bias_t[:, 5] = bqg[0:128]

        hsT = np.ascontiguousarray(
            hs[b].T.reshape(NKT, 128, NNT, 512).transpose(1, 2, 0, 3)).astype(bf16)

        maps.append({
            "hsT": hsT,
            "w5": w5_t,
            "wqg01": wtiles(Wqg[:, 0:128]),
            "wvvg": wtiles(np.concatenate([Wv, Wvg], axis=1)),
            "bvvg": np.concatenate([bv, bvg])[None, :].astype(bf16),
            "bias_t": bias_t,
            "masks": masks,
            "id96": id96,
        })
    return maps


def kernel(**inputs):
    g = int(np.asarray(inputs["num_global"]))
    assert g == G, f"kernel compiled for num_global=32, got {g}"
    if "nc" not in _cache:
        _cache["nc"] = _build()
    nc = _cache["nc"]
    in_maps = _prep_inputs(inputs)
    res = run_bass_kernel_spmd(nc, in_maps, list(range(8)))
    return assemble(res.results)


def assemble(results):
    out = np.zeros((B, S, D), np.float32)
    for c in range(8):
        b, hg = c // 4, c % 4
        o = results[c]["out"]          # [3, 65, S]
        og = results[c]["outg"]        # [3, G, 65]
        for h in range(3):
            col = 192 * hg + 64 * h
            out[b, :, col:col + 64] = (o[h, 0:64] / o[h, 64]).T
            out[b, 0:G, col:col + 64] = og[h, :, 0:64] / og[h, :, 64:65]
    return out
